# revision 1
# baseline (speedup 1.0000x reference)
"""Trainium2 Bass kernel for DiffSelfAttention (B=1, T=2048, C=2048, 16 v-heads).

Sharding: tensor-parallel over heads across 8 NeuronCores. Core c owns
v-heads {2c, 2c+1} plus the matching q/k heads of both differential branches.
Each core computes its qkv slice, the attention for its 4 q/k heads, the
differential + per-head RMSNorm, and a partial projection
y_c = out_c @ w_proj[rows_c]. The host sums the 8 partials (unshard step).

Layout/strategy notes:
  - All matmuls run as float32r (full-rate fp32 on the PE at N>=256,
    ~2e-4 element rounding). DMA loads go directly into fp32r tiles;
    on-chip fp32r operands are produced by compute ops (engines round on
    write), which is what the BIR verifier requires.
  - q/k are produced directly transposed ([d, T]); v in natural layout
    ([T, d]); scores computed transposed ([tk, tq]) so probs@v needs no
    transposes anywhere.
  - Softmax divisions are eliminated: RMSNorm is invariant to any
    per-column positive scale, so instead of a1/r1 - lam*a2/r2 we feed it
    o' = a1*r2 - lam*a2*r1 (r = exp-sum broadcasts from a ones-matmul).
    The 1e-6 RMS eps is dropped: mean(o'^2) >> eps always for this data.
  - rsqrt for RMS is computed as exp(-0.5*log(m)) on the ACT engine
    (Reciprocal/Rsqrt activations are banned; Log+Exp share one ACT
    table set so there are no mid-kernel table switches).
  - Softmax column sums use two interleaved DVE accumulator chains so the
    serial dependency never gates the ACT exp stream.
"""

import math

import numpy as np

import concourse.bass as bass
import concourse.bacc as bacc
import concourse.mybir as mybir
import concourse.tile as tile

F32 = mybir.dt.float32
F32R = mybir.dt.float32r

T = 2048
C = 2048
N_HEAD = 16
H_DIM = 64
D2 = 2 * H_DIM  # 128 (v-head dim, also the RMS group size)
LAMBDA_INIT = 0.8 - 0.6 * math.exp(-0.3)
SCALE = 1.0 / math.sqrt(H_DIM)
P = 128
KSLABS = C // P  # 16 contraction slabs
TT = T // P  # 16 t-tiles
NCH = 512  # moving-operand chunk (max for 4-byte dtypes)
HQ = T // 2  # 1024-wide tq halves in the attention inner loop
N_CORES = 8

EXP = mybir.ActivationFunctionType.Exp
LOG = mybir.ActivationFunctionType.Ln
MULT = mybir.AluOpType.mult
ADD = mybir.AluOpType.add


def build(lam: float) -> bass.Bass:
    nc = bacc.Bacc("TRN2", target_bir_lowering=False, debug=False)

    xt_d = nc.dram_tensor("xt", [P, 4, KSLABS, NCH], F32R, kind="ExternalInput")
    wqk_d = nc.dram_tensor("wqk", [P, KSLABS, 4 * P], F32R, kind="ExternalInput")
    wv_d = nc.dram_tensor("wv", [P, KSLABS, 2 * D2], F32R, kind="ExternalInput")
    wp_d = nc.dram_tensor("wp", [P, 2, T], F32R, kind="ExternalInput")
    sv_d = nc.dram_tensor("sv", [P, 1], F32, kind="ExternalInput")
    y_d = nc.dram_tensor("y", [TT, P, T], F32, kind="ExternalOutput")

    with tile.TileContext(nc) as tc:
        with tc.tile_pool(name="persist", bufs=1) as persist:
            sv = persist.tile([P, 1], F32)
            ones_f = persist.tile([P, P], F32)
            ones = persist.tile([P, P], F32R)
            qk = persist.tile([P, 4, T], F32R)  # q1|q2|k1|k2, [d, T] layout
            vnat = persist.tile([P, TT, 2 * D2], F32R)  # v, [T, d] layout
            nc.sync.dma_start(out=sv, in_=sv_d[:])
            nc.vector.memset(ones_f, 1.0)
            nc.vector.tensor_copy(ones, ones_f)

            # ---------- phase 1: qkv projections ----------
            with tc.tile_pool(name="w1", bufs=1) as w1p, \
                 tc.tile_pool(name="xt", bufs=2) as xtp, \
                 tc.tile_pool(name="ps_qk", bufs=2, space="PSUM") as pqk, \
                 tc.tile_pool(name="ps_v", bufs=2, space="PSUM") as pvp:
                wqk = w1p.tile([P, KSLABS, 4 * P], F32R)
                wv = w1p.tile([P, KSLABS, 2 * D2], F32R)
                nc.sync.dma_start(out=wqk, in_=wqk_d[:])
                nc.sync.dma_start(out=wv, in_=wv_d[:])
                for n in range(T // NCH):  # 512-wide t chunks
                    xt = xtp.tile([P, KSLABS, NCH], F32R)
                    nc.sync.dma_start(out=xt, in_=xt_d[:, n, :, :])
                    for m in range(4):  # q1, q2, k1, k2
                        ps = pqk.tile([P, NCH], F32)
                        for k in range(KSLABS):
                            nc.tensor.matmul(
                                ps,
                                wqk[:, k, m * P:(m + 1) * P],
                                xt[:, k, :],
                                start=(k == 0),
                                stop=(k == KSLABS - 1),
                            )
                        nc.vector.tensor_copy(qk[:, m, n * NCH:(n + 1) * NCH], ps)
                    for t2 in range(NCH // P):  # t-tiles in this chunk
                        ps = pvp.tile([P, 2 * D2], F32)
                        for k in range(KSLABS):
                            nc.tensor.matmul(
                                ps,
                                xt[:, k, t2 * P:(t2 + 1) * P],
                                wv[:, k, :],
                                start=(k == 0),
                                stop=(k == KSLABS - 1),
                            )
                        nc.vector.tensor_copy(vnat[:, n * (NCH // P) + t2, :], ps)

            # ---------- phases 2+3 ----------
            with tc.tile_pool(name="wp", bufs=1) as wpp:
                wp = wpp.tile([P, 2, T], F32R)
                on = wpp.tile([P, 2, T], F32R)  # normed diff out, [d, T] per vh
                nc.sync.dma_start(out=wp, in_=wp_d[:])

                # ---------- phase 2: attention ----------
                # Both v-head streams (array rows 0-63 / 64-127) are packed
                # into shared [P, 2, NCH] tiles: one ACT exp covers both, and
                # the PE gets 6 matmuls per tk-slab (scores x2, pv x2,
                # colsum x2) so it never idles long enough for the HAM
                # clock-gate to re-throttle it to 1.2 GHz.
                with tc.tile_pool(name="ps_s", bufs=2, space="PSUM") as psp, \
                     tc.tile_pool(name="ps_a", bufs=1, space="PSUM") as pap, \
                     tc.tile_pool(name="ps_r", bufs=1, space="PSUM") as rp, \
                     tc.tile_pool(name="exp", bufs=4) as ep, \
                     tc.tile_pool(name="keep", bufs=1) as kp:
                    opk = kp.tile([P, 2, T], F32)  # scaled diff o', per vh
                    a1u = {}
                    r1l = {}
                    for br in range(2):
                        for q4 in range(4):  # 512-wide tq quarters
                            c0 = q4 * NCH
                            pa = pap.tile([P, 2, NCH], F32, tag="pa")
                            r = rp.tile([P, 2, NCH], F32, tag="r")
                            for k in range(TT):  # tk slabs
                                ps = psp.tile([P, 2, NCH], F32, tag="s")
                                et = ep.tile([P, 2, NCH], F32R, tag="er")
                                for vh in range(2):
                                    rows = slice(vh * H_DIM, (vh + 1) * H_DIM)
                                    nc.tensor.matmul(
                                        ps[:, vh, :],
                                        qk[rows, 2 + br, k * P:(k + 1) * P],
                                        qk[rows, br, c0:c0 + NCH],
                                        start=True,
                                        stop=True,
                                    )
                                nc.scalar.activation(et, ps, EXP, scale=SCALE)
                                for vh in range(2):
                                    nc.tensor.matmul(
                                        pa[:, vh, :],
                                        vnat[:, k, vh * D2:(vh + 1) * D2],
                                        et[:, vh, :],
                                        start=(k == 0),
                                        stop=(k == TT - 1),
                                    )
                                    nc.tensor.matmul(
                                        r[:, vh, :],
                                        ones,
                                        et[:, vh, :],
                                        start=(k == 0),
                                        stop=(k == TT - 1),
                                    )
                            if br == 0:
                                # keep unnormalized a1 and -lam*r1 for branch 2
                                a1u[q4] = kp.tile([P, 2, NCH], F32, tag=f"a1u{q4}", name=f"a1u{q4}")
                                nc.vector.tensor_copy(a1u[q4], pa)
                                r1l[q4] = kp.tile([P, 2, NCH], F32, tag=f"r1l{q4}", name=f"r1l{q4}")
                                nc.vector.tensor_scalar_mul(r1l[q4], r, -lam)
                            else:
                                # o' = a1*r2 - lam*a2*r1  (a per-column positive
                                # rescale of o; RMSNorm cancels it)
                                m1 = ep.tile([P, 2, NCH], F32, tag="m1")
                                nc.vector.tensor_mul(m1, a1u[q4], r)
                                m2 = ep.tile([P, 2, NCH], F32, tag="m2")
                                nc.vector.tensor_mul(m2, pa, r1l[q4])
                                nc.vector.tensor_add(opk[:, :, c0:c0 + NCH], m1, m2)
                    # RMS: rsqrt(mean o'^2) = exp(-0.5*ln(mean)). All Ln ops
                    # emitted before all Exp ops -> at most 2 ACT table loads.
                    psms = []
                    for vh in range(2):
                        for hf in range(2):
                            q0 = hf * HQ
                            sq = ep.tile([P, HQ], F32R, tag="er")
                            nc.vector.tensor_mul(sq, opk[:, vh, q0:q0 + HQ], opk[:, vh, q0:q0 + HQ])
                            psm = psp.tile([P, HQ], F32, tag="s")
                            for c2 in range(2):
                                nc.tensor.matmul(
                                    psm[:, c2 * NCH:(c2 + 1) * NCH],
                                    ones,
                                    sq[:, c2 * NCH:(c2 + 1) * NCH],
                                    start=True,
                                    stop=True,
                                )
                            ln = kp.tile([P, HQ], F32, tag=f"a1u{2 * vh + hf}", name=f"ln{vh}{hf}")
                            nc.scalar.activation(ln, psm, LOG, scale=1.0 / D2)
                            psms.append(ln)
                    for vh in range(2):
                        for hf in range(2):
                            q0 = hf * HQ
                            rsq = ep.tile([P, HQ], F32, tag="m1")
                            nc.scalar.activation(rsq, psms[2 * vh + hf], EXP, scale=-0.5)
                            nc.vector.scalar_tensor_tensor(
                                on[:, vh, q0:q0 + HQ],
                                opk[:, vh, q0:q0 + HQ],
                                sv, rsq, op0=MULT, op1=MULT,
                            )

                # ---------- phase 3: output projection (partial sum) ----------
                with tc.tile_pool(name="ps_y", bufs=4, space="PSUM") as pyp, \
                     tc.tile_pool(name="ysb", bufs=3) as yp:
                    for tt_i in range(TT):
                        ysb = yp.tile([P, T], F32)
                        for nch in range(T // NCH):
                            py = pyp.tile([P, NCH], F32)
                            for vh in range(2):
                                nc.tensor.matmul(
                                    py,
                                    on[:, vh, tt_i * P:(tt_i + 1) * P],
                                    wp[:, vh, nch * NCH:(nch + 1) * NCH],
                                    start=(vh == 0),
                                    stop=(vh == 1),
                                )
                            nc.vector.tensor_copy(ysb[:, nch * NCH:(nch + 1) * NCH], py)
                        nc.sync.dma_start(out=y_d[tt_i], in_=ysb)
    nc.finalize()
    return nc


def _core_inputs(x, w_qkv, w_proj, rms_scale):
    """Host-side shard prep: per-core weight slices + replicated x^T."""
    xt = np.ascontiguousarray(x.reshape(T, C).T)  # [C, T]
    xtr = np.ascontiguousarray(
        xt.reshape(KSLABS, P, T // NCH, NCH).transpose(1, 2, 0, 3)
    )
    sv = np.ascontiguousarray(
        (rms_scale.astype(np.float32) * np.float32(1.0 - LAMBDA_INIT)).reshape(P, 1)
    )
    maps = []
    for c in range(N_CORES):
        cols = [
            w_qkv[:, 0 * 1024 + c * P:0 * 1024 + (c + 1) * P],  # q1 heads 2c,2c+1
            w_qkv[:, 1 * 1024 + c * P:1 * 1024 + (c + 1) * P],  # q2
            w_qkv[:, 2 * 1024 + c * P:2 * 1024 + (c + 1) * P],  # k1
            w_qkv[:, 3 * 1024 + c * P:3 * 1024 + (c + 1) * P],  # k2
        ]
        wqk = np.concatenate(cols, axis=1)  # [C, 512]
        wqk = np.ascontiguousarray(wqk.reshape(KSLABS, P, 4 * P).transpose(1, 0, 2))
        wv = w_qkv[:, 2 * C + c * 2 * D2:2 * C + (c + 1) * 2 * D2]  # [C, 256]
        wv = np.ascontiguousarray(wv.reshape(KSLABS, P, 2 * D2).transpose(1, 0, 2))
        wp = w_proj[c * 2 * D2:(c + 1) * 2 * D2, :]  # [256, T]
        wp = np.ascontiguousarray(wp.reshape(2, P, T).transpose(1, 0, 2))
        maps.append({"xt": xtr, "wqk": wqk, "wv": wv, "wp": wp, "sv": sv})
    return maps


def kernel(x, w_qkv, w_proj, lambda_q1, lambda_k1, lambda_q2, lambda_k2, rms_scale):
    from concourse.bass_utils import run_bass_kernel_spmd

    x = np.asarray(x, dtype=np.float32)
    w_qkv = np.asarray(w_qkv, dtype=np.float32)
    w_proj = np.asarray(w_proj, dtype=np.float32)
    rms_scale = np.asarray(rms_scale, dtype=np.float32)
    lam1 = np.exp(np.sum(np.asarray(lambda_q1) * np.asarray(lambda_k1), dtype=np.float32))
    lam2 = np.exp(np.sum(np.asarray(lambda_q2) * np.asarray(lambda_k2), dtype=np.float32))
    lam = float(lam1 - lam2 + LAMBDA_INIT)

    nc = build(lam)
    in_maps = _core_inputs(x, w_qkv, w_proj, rms_scale)
    res = run_bass_kernel_spmd(nc, in_maps, core_ids=list(range(N_CORES)))
    y = np.zeros((TT, P, T), np.float32)
    for rmap in res.results:
        y += rmap["y"]
    return y.reshape(1, T, C)



# revision 8
# speedup vs baseline: 1.2826x; 1.2826x over previous
"""Trainium2 Bass kernel for DiffSelfAttention (B=1, T=2048, C=2048, 16 v-heads).

Sharding: tensor-parallel over heads across 8 NeuronCores. Core c owns
v-heads {2c, 2c+1} plus the matching q/k heads of both differential branches.
Each core computes its qkv slice, the attention for its 4 q/k heads, the
differential + per-head RMSNorm, and a partial projection
y_c = out_c @ w_proj[rows_c]. The host sums the 8 partials (unshard step).

Perf-driven layout (v2, from NTFF trace analysis of the fp32r baseline):
  - bf16 operands everywhere on-chip (fp32 PSUM accumulation). Halves all
    DMA and SBUF traffic; PE streams 1 col/cycle either way.
  - Input DMA is ordered wv -> xt chunk0 -> wqk so the first matmuls
    start ~8us in instead of ~40us (baseline stalled on one big serial
    weight load).
  - Phase 2: scores PSUM pool is 3-deep so the scores->exp->pv chain
    never idles the PE (baseline's 2-deep pool cost ~1us per 2 k-slabs).
  - Softmax denominators (colsum of exp) are deferred out of the k-loop:
    exp tiles are kept in SBUF for a whole tq-chunk, then summed either
    with fp8e4 DoubleRow matmuls (0.5 cyc/row; r is a positive sum so
    fp8 quantization averages out to ~0.2%) or bf16 matmuls (fallback).
  - Softmax divisions eliminated: RMSNorm is invariant to per-column
    positive scales, so o' = a1*r2 - lam*a2*r1 feeds the norm directly.
  - rsqrt = exp(-0.5*ln(m)) on ACT (Ln+Exp share one table set).
  - RMSNorm is fused per 512-wide tq-chunk; projection runs at the end
    with stationary reuse (on-tile loaded once per (tt,vh), 4 matmuls),
    y written as bf16 to halve the output DMA.
"""

import math

import numpy as np

import concourse.bass as bass
import concourse.bacc as bacc
import concourse.mybir as mybir
import concourse.tile as tile

F32 = mybir.dt.float32
BF16 = mybir.dt.bfloat16
FP8 = mybir.dt.float8e4

T = 2048
C = 2048
N_HEAD = 16
H_DIM = 64
D2 = 2 * H_DIM  # 128 (v-head dim, also the RMS group size)
LAMBDA_INIT = 0.8 - 0.6 * math.exp(-0.3)
SCALE = 1.0 / math.sqrt(H_DIM)
P = 128
KSLABS = C // P  # 16 contraction slabs
TT = T // P  # 16 t-tiles
NCH = 512  # qkv-phase moving-operand chunk
NQ = 512  # attention tq chunk width
N_CORES = 8

USE_FP8_COLSUM = False

EXP = mybir.ActivationFunctionType.Exp
LOG = mybir.ActivationFunctionType.Ln
MULT = mybir.AluOpType.mult
ADD = mybir.AluOpType.add
DR = mybir.MatmulPerfMode.DoubleRow


def build(lam: float) -> bass.Bass:
    nc = bacc.Bacc("TRN2", target_bir_lowering=False, debug=False)

    xt_d = nc.dram_tensor("xt", [P, T // NCH, KSLABS, NCH], BF16, kind="ExternalInput")
    wqk_d = nc.dram_tensor("wqk", [P, KSLABS, 4 * P], BF16, kind="ExternalInput")
    wv_d = nc.dram_tensor("wv", [P, KSLABS, 2 * D2], BF16, kind="ExternalInput")
    wp_d = nc.dram_tensor("wp", [P, 2, T], BF16, kind="ExternalInput")
    sv_d = nc.dram_tensor("sv", [P, 1], F32, kind="ExternalInput")
    y_d = nc.dram_tensor("y", [TT, P, T], BF16, kind="ExternalOutput")

    with tile.TileContext(nc) as tc:
        with tc.tile_pool(name="persist", bufs=1) as persist:
            sv = persist.tile([P, 1], F32)
            ones_f = persist.tile([P, P], F32)
            ones = persist.tile([P, P], BF16)
            qk = persist.tile([P, 4, T], BF16)  # q1|q2|k1|k2, [d, T] layout
            vnat = persist.tile([P, TT, 2 * D2], BF16)  # v, [T, d] layout
            on = persist.tile([P, 2, T], BF16)  # normed diff out, [d, T] per vh
            nc.sync.dma_start(out=sv, in_=sv_d[:])
            nc.vector.memset(ones_f, 1.0)
            nc.vector.tensor_copy(ones, ones_f)
            if USE_FP8_COLSUM:
                ones8 = persist.tile([P, 2, P], FP8)
                nc.vector.memset(ones8, 1.0)

            # ---------- phase 1: qkv projections ----------
            with tc.tile_pool(name="w1", bufs=1) as w1p, \
                 tc.tile_pool(name="xt", bufs=2) as xtp, \
                 tc.tile_pool(name="ps_qk", bufs=4, space="PSUM") as pqk, \
                 tc.tile_pool(name="ps_v", bufs=4, space="PSUM") as pvp:
                wv = w1p.tile([P, KSLABS, 2 * D2], BF16)
                wqk = w1p.tile([P, KSLABS, 4 * P], BF16)
                nc.sync.dma_start(out=wv, in_=wv_d[:])
                xts = []
                for n in range(T // NCH):
                    xts.append(
                        xtp.tile([P, KSLABS, NCH], BF16, tag="xt", name=f"xt{n}")
                    )
                nc.gpsimd.dma_start(out=xts[0], in_=xt_d[:, 0, :, :])
                nc.sync.dma_start(out=wqk, in_=wqk_d[:])
                for n in range(T // NCH):  # 512-wide t chunks
                    xt = xts[n]
                    if n + 1 < T // NCH:
                        nc.gpsimd.dma_start(out=xts[n + 1], in_=xt_d[:, n + 1, :, :])
                    for t2 in range(NCH // P):  # v first: needs only wv
                        ps = pvp.tile([P, 2 * D2], F32)
                        for k in range(KSLABS):
                            nc.tensor.matmul(
                                ps,
                                xt[:, k, t2 * P:(t2 + 1) * P],
                                wv[:, k, :],
                                start=(k == 0),
                                stop=(k == KSLABS - 1),
                            )
                        nc.vector.tensor_copy(vnat[:, n * (NCH // P) + t2, :], ps)
                    for m in range(4):  # q1, q2, k1, k2
                        ps = pqk.tile([P, NCH], F32)
                        for k in range(KSLABS):
                            nc.tensor.matmul(
                                ps,
                                wqk[:, k, m * P:(m + 1) * P],
                                xt[:, k, :],
                                start=(k == 0),
                                stop=(k == KSLABS - 1),
                            )
                        nc.vector.tensor_copy(qk[:, m, n * NCH:(n + 1) * NCH], ps)

            # ---------- phase 2: attention + rms, per 512-wide tq chunk ----------
            with tc.tile_pool(name="wp", bufs=1) as wpp, \
                 tc.tile_pool(name="eta", bufs=2) as etap, \
                 tc.tile_pool(name="keep", bufs=2) as kp, \
                 tc.tile_pool(name="sml", bufs=2) as smp:
                wp = wpp.tile([P, 2, T], BF16)
                nc.sync.dma_start(out=wp, in_=wp_d[:])
                for q4 in range(T // NQ):  # tq chunks
                    c0 = q4 * NQ
                    et = etap.tile([P, KSLABS, 2, NQ], BF16, tag="et")
                    if USE_FP8_COLSUM:
                        et8 = etap.tile([P, 2, KSLABS // 2, 2, NQ], FP8, tag="et8")
                    a1u = kp.tile([P, 2, NQ], F32, tag="a1u")
                    r1s = kp.tile([P, 2, NQ], F32, tag="r1s")
                    opk = kp.tile([P, 2, NQ], F32, tag="opk")
                    for br in range(2):
                        with tc.tile_pool(name="pa", bufs=1, space="PSUM") as pap:
                            pa = pap.tile([P, 2, NQ], F32)
                            with tc.tile_pool(name="ps", bufs=3, space="PSUM") as psp:
                                for k in range(KSLABS):
                                    ps = psp.tile([P, 2, NQ], F32, tag="s")
                                    for vh in range(2):
                                        rows = slice(vh * H_DIM, (vh + 1) * H_DIM)
                                        nc.tensor.matmul(
                                            ps[:, vh, :],
                                            qk[rows, 2 + br, k * P:(k + 1) * P],
                                            qk[rows, br, c0:c0 + NQ],
                                            start=True,
                                            stop=True,
                                        )
                                    nc.scalar.activation(
                                        et[:, k, :, :], ps, EXP, scale=SCALE
                                    )
                                    if USE_FP8_COLSUM:
                                        nc.vector.tensor_copy(
                                            et8[:, :, k // 2, k % 2, :], et[:, k, :, :]
                                        )
                                    for vh in range(2):
                                        nc.tensor.matmul(
                                            pa[:, vh, :],
                                            vnat[:, k, vh * D2:(vh + 1) * D2],
                                            et[:, k, vh, :],
                                            start=(k == 0),
                                            stop=(k == KSLABS - 1),
                                        )
                            # exp column sums (deferred): r = colsum(et)
                            with tc.tile_pool(name="r", bufs=1, space="PSUM") as rp:
                                r = rp.tile([P, 2, NQ], F32)
                                if USE_FP8_COLSUM:
                                    for vh in range(2):
                                        for pr in range(KSLABS // 2):
                                            nc.tensor.matmul(
                                                r[:, vh, :],
                                                ones8,
                                                et8[:, vh, pr, :, :],
                                                start=(pr == 0),
                                                stop=(pr == KSLABS // 2 - 1),
                                                perf_mode=DR,
                                            )
                                else:
                                    for vh in range(2):
                                        for k in range(KSLABS):
                                            nc.tensor.matmul(
                                                r[:, vh, :],
                                                ones,
                                                et[:, k, vh, :],
                                                start=(k == 0),
                                                stop=(k == KSLABS - 1),
                                            )
                                if br == 0:
                                    nc.vector.tensor_copy(a1u, pa)
                                    nc.vector.tensor_copy(r1s, r)
                                else:
                                    # o' = a1*r2 - lam*a2*r1 (per-column positive
                                    # rescale of o; RMSNorm cancels it)
                                    m1 = smp.tile([P, 2, NQ], F32, tag="m1")
                                    nc.vector.tensor_mul(m1, a1u, r)
                                    m2 = smp.tile([P, 2, NQ], F32, tag="m2")
                                    nc.vector.tensor_mul(m2, pa, r1s)
                                    nc.vector.scalar_tensor_tensor(
                                        opk, m2, -lam, m1, op0=MULT, op1=ADD,
                                    )
                    # per-chunk RMS: rsqrt(mean o'^2) = exp(-0.5*ln(mean))
                    sq = smp.tile([P, 2, NQ], BF16, tag="sq")
                    nc.vector.tensor_mul(sq, opk, opk)
                    with tc.tile_pool(name="psm", bufs=1, space="PSUM") as pmp:
                        psm = pmp.tile([P, 2, NQ], F32)
                        for vh in range(2):
                            nc.tensor.matmul(
                                psm[:, vh, :],
                                ones,
                                sq[:, vh, :],
                                start=True,
                                stop=True,
                            )
                        lnv = smp.tile([P, 2, NQ], F32, tag="m1")
                        nc.scalar.activation(lnv, psm, LOG, scale=1.0 / D2)
                    rsq = smp.tile([P, 2, NQ], F32, tag="m2")
                    nc.scalar.activation(rsq, lnv, EXP, scale=-0.5)
                    nc.vector.scalar_tensor_tensor(
                        on[:, :, c0:c0 + NQ], opk, sv, rsq, op0=MULT, op1=MULT,
                    )

                # ---------- phase 3: output projection (partial sum) ----------
                with tc.tile_pool(name="ps_y", bufs=8, space="PSUM") as pyp, \
                     tc.tile_pool(name="ysb", bufs=3) as yp:
                    for tt_i in range(TT):
                        ysb = yp.tile([P, T], BF16)
                        pys = [
                            pyp.tile([P, NCH], F32, tag="py", name=f"py{i}")
                            for i in range(4)
                        ]
                        for vh in range(2):
                            for nchi in range(4):
                                nc.tensor.matmul(
                                    pys[nchi],
                                    on[:, vh, tt_i * P:(tt_i + 1) * P],
                                    wp[:, vh, nchi * NCH:(nchi + 1) * NCH],
                                    start=(vh == 0),
                                    stop=(vh == 1),
                                )
                        for nchi in range(4):
                            nc.any.tensor_copy(
                                ysb[:, nchi * NCH:(nchi + 1) * NCH], pys[nchi]
                            )
                        nc.sync.dma_start(out=y_d[tt_i], in_=ysb)
    nc.finalize()
    return nc


def _core_inputs(x, w_qkv, w_proj, rms_scale):
    """Host-side shard prep: per-core weight slices + replicated x^T (bf16)."""
    import ml_dtypes

    bf16 = ml_dtypes.bfloat16
    xt = np.ascontiguousarray(x.reshape(T, C).T)  # [C, T]
    xtr = np.ascontiguousarray(
        xt.reshape(KSLABS, P, T // NCH, NCH).transpose(1, 2, 0, 3)
    ).astype(bf16)
    sv = np.ascontiguousarray(
        (rms_scale.astype(np.float32) * np.float32(1.0 - LAMBDA_INIT)).reshape(P, 1)
    )
    maps = []
    for c in range(N_CORES):
        cols = [
            w_qkv[:, 0 * 1024 + c * P:0 * 1024 + (c + 1) * P],  # q1 heads 2c,2c+1
            w_qkv[:, 1 * 1024 + c * P:1 * 1024 + (c + 1) * P],  # q2
            w_qkv[:, 2 * 1024 + c * P:2 * 1024 + (c + 1) * P],  # k1
            w_qkv[:, 3 * 1024 + c * P:3 * 1024 + (c + 1) * P],  # k2
        ]
        wqk = np.concatenate(cols, axis=1)  # [C, 512]
        wqk = np.ascontiguousarray(
            wqk.reshape(KSLABS, P, 4 * P).transpose(1, 0, 2)
        ).astype(bf16)
        wv = w_qkv[:, 2 * C + c * 2 * D2:2 * C + (c + 1) * 2 * D2]  # [C, 256]
        wv = np.ascontiguousarray(
            wv.reshape(KSLABS, P, 2 * D2).transpose(1, 0, 2)
        ).astype(bf16)
        wp = w_proj[c * 2 * D2:(c + 1) * 2 * D2, :]  # [256, T]
        wp = np.ascontiguousarray(wp.reshape(2, P, T).transpose(1, 0, 2)).astype(bf16)
        maps.append({"xt": xtr, "wqk": wqk, "wv": wv, "wp": wp, "sv": sv})
    return maps


def kernel(x, w_qkv, w_proj, lambda_q1, lambda_k1, lambda_q2, lambda_k2, rms_scale):
    from concourse.bass_utils import run_bass_kernel_spmd

    x = np.asarray(x, dtype=np.float32)
    w_qkv = np.asarray(w_qkv, dtype=np.float32)
    w_proj = np.asarray(w_proj, dtype=np.float32)
    rms_scale = np.asarray(rms_scale, dtype=np.float32)
    lam1 = np.exp(np.sum(np.asarray(lambda_q1) * np.asarray(lambda_k1), dtype=np.float32))
    lam2 = np.exp(np.sum(np.asarray(lambda_q2) * np.asarray(lambda_k2), dtype=np.float32))
    lam = float(lam1 - lam2 + LAMBDA_INIT)

    nc = build(lam)
    in_maps = _core_inputs(x, w_qkv, w_proj, rms_scale)
    res = run_bass_kernel_spmd(nc, in_maps, core_ids=list(range(N_CORES)))
    y = np.zeros((TT, P, T), np.float32)
    for rmap in res.results:
        y += rmap["y"].astype(np.float32)
    return y.reshape(1, T, C)
